# revision 1
# baseline (speedup 1.0000x reference)
"""HONU order-3 kernel for 8 TRN2 NeuronCores.

Math: out[b] = sum_{i<=j<=k} w_ijk * xf_i * xf_j * xf_k,  xf = [1, x] (127 feats).

Restructuring: group combos by pair (i,j) (lex order => per-pair weights are a
contiguous slice of `weights`).  Let W[(i,j), k] = w_ijk for k>=j (0 otherwise).
Then  Z[b,(i,j)] = sum_k W[(i,j),k] * xf[b,k]   (a dense matmul), and
      out[b]     = sum_i xf_i * sum_{j>=i} xf_j * Z[b,(i,j)]
which maps onto one fused op per i-row (scalar_tensor_tensor):
      accum = sum_j ((Z * xf_i) * xf_j).

Sharding: pair-rows i are dealt round-robin to the 8 cores (core c gets rows
i = 8t + c, t = 0..15), so every core runs the same (SPMD) program: 16 fused
ops per 128-batch tile whose widths are padded to the 8-aligned grid
(row i covers j in [8*floor(i/8), 128); padding columns carry zero weights).
The fused ops are split between DVE and GPSIMD; ACT stages Z from PSUM to
SBUF (GPSIMD cannot read PSUM).  x is replicated; each core returns a [256,1]
partial that the host sums.

Matmuls run in float32r (full-rate fp32 PE mode); flip MM_F32R=False for
exact-fp32 (4x slower PE) if precision ever regresses.
"""

import numpy as np

import concourse.bass as bass
import concourse.bacc as bacc
import concourse.tile as tile
import concourse.mybir as mybir
from concourse.bass_utils import run_bass_kernel_spmd

F32 = mybir.dt.float32
F32R = mybir.dt.float32r
MM_F32R = True

P = 128
NF = 127            # features incl. bias
B = 256             # batch
NCLASS = 16         # width classes (i-rows per core)
WIDTHS = [128 - 8 * t for t in range(NCLASS)]           # 128,120,...,8
OFFS = np.concatenate([[0], np.cumsum(WIDTHS)])          # class col offsets
NCOLS = int(OFFS[-1])                                    # 1088
# chunk = (class range); each chunk is one matmul (N<=512)
CHUNKS = [(0, 4), (4, 9), (9, 16)]
CHUNK_COLS = [int(OFFS[hi] - OFFS[lo]) for lo, hi in CHUNKS]  # 464, 400, 224
GPS_CLASSES = set()   # GPSIMD cannot run TensorScalarPtr (walrus engine check)

_CACHE = {}


def _build_nc():
    mm_dt = F32R if MM_F32R else F32
    nc = bacc.Bacc("TRN2", target_bir_lowering=False, debug=False)
    xt = nc.dram_tensor("xt", [P, B], mm_dt, kind="ExternalInput")    # xf^T padded
    xb = nc.dram_tensor("xb", [B, P], F32, kind="ExternalInput")      # xf padded
    xs = nc.dram_tensor("xs", [B, NCLASS], F32, kind="ExternalInput")  # xf_i per class
    wds = [
        nc.dram_tensor(f"wd{ci}", [P, n], mm_dt, kind="ExternalInput")
        for ci, n in enumerate(CHUNK_COLS)
    ]
    out = nc.dram_tensor("out", [B, 1], F32, kind="ExternalOutput")

    with tile.TileContext(nc) as tc:
        with (
            tc.tile_pool(name="const", bufs=1) as cpool,
            tc.tile_pool(name="sb", bufs=2) as sb,
            tc.tile_pool(name="scrv", bufs=2) as scrv,
            tc.tile_pool(name="scrg", bufs=2) as scrg,
            tc.tile_pool(name="ps", bufs=2, space="PSUM") as ps,
        ):
            # spread loads over four HWDGE queues so the first matmul's
            # inputs (xt + wd0) land as early as possible
            xt_t = cpool.tile([P, B], mm_dt, tag="xt")
            nc.sync.dma_start(xt_t[:], xt[:])
            wd_t = [cpool.tile([P, n], mm_dt, tag=f"wd{ci}", name=f"wd{ci}_t")
                    for ci, n in enumerate(CHUNK_COLS)]
            nc.scalar.dma_start(wd_t[0][:], wds[0][:])
            nc.scalar.dma_start(wd_t[1][:], wds[1][:])
            nc.scalar.dma_start(wd_t[2][:], wds[2][:])
            xb_ts, xs_ts = [], []
            for bt in range(2):
                xb_t = sb.tile([P, P], F32, tag=f"xb{bt}", name=f"xb{bt}_t")
                nc.sync.dma_start(xb_t[:], xb[bt * P:(bt + 1) * P, :])
                xs_t = sb.tile([P, NCLASS], F32, tag=f"xs{bt}", name=f"xs{bt}_t")
                nc.sync.dma_start(xs_t[:], xs[bt * P:(bt + 1) * P, :])
                xb_ts.append(xb_t)
                xs_ts.append(xs_t)

            for bt in range(2):
                xb_t, xs_t = xb_ts[bt], xs_ts[bt]
                g = sb.tile([P, NCLASS], F32, tag=f"g{bt}", name=f"g{bt}_t")
                for ci, (lo, hi) in enumerate(CHUNKS):
                    n = CHUNK_COLS[ci]
                    z_ps = ps.tile([P, n], F32, tag=f"z{ci}", name=f"z{ci}_ps")
                    nc.tensor.matmul(
                        z_ps[:], xt_t[:, bt * P:(bt + 1) * P], wd_t[ci][:],
                        start=True, stop=True,
                    )
                    z_sb = sb.tile([P, n], F32, tag=f"zsb{ci}", name=f"z{ci}_sb")
                    nc.scalar.copy(z_sb[:], z_ps[:])
                    for t in range(lo, hi):
                        w = WIDTHS[t]
                        o = int(OFFS[t] - OFFS[lo])
                        eng = nc.gpsimd if t in GPS_CLASSES else nc.vector
                        pool = scrg if t in GPS_CLASSES else scrv
                        s = pool.tile([P, 128], F32, tag="s", name="s_t")
                        eng.scalar_tensor_tensor(
                            out=s[:, :w],
                            in0=z_sb[:, o:o + w],
                            scalar=xs_t[:, t:t + 1],
                            in1=xb_t[:, 8 * t:8 * t + w],
                            op0=mybir.AluOpType.mult,
                            op1=mybir.AluOpType.mult,
                            accum_out=g[:, t:t + 1],
                        )
                res = sb.tile([P, 1], F32, tag=f"res{bt}", name=f"res{bt}_t")
                nc.vector.reduce_sum(res[:], g[:], axis=mybir.AxisListType.X)
                nc.sync.dma_start(out[bt * P:(bt + 1) * P, :], res[:])
    nc.compile()
    return nc


def _prep_inputs(x, weights, comb_idx):
    """Host-side layout prep (no FLOPs on the runtime data beyond zero-fill
    scatter): build xf paddings and the per-core dense weight chunks."""
    x = np.ascontiguousarray(np.asarray(x, dtype=np.float32))
    w = np.asarray(weights, dtype=np.float32).ravel()
    ci = np.asarray(comb_idx)
    i_, j_ = ci[:, 0].astype(np.int64), ci[:, 1].astype(np.int64)
    k_ = ci[:, 2].astype(np.int64)

    xf = np.concatenate([np.ones((B, 1), np.float32), x], axis=1)   # [256,127]
    xb = np.zeros((B, P), np.float32)
    xb[:, :NF] = xf
    xt = np.zeros((P, B), np.float32)
    xt[:NF, :] = xf.T

    # lex pair-row index of each combo
    ar = np.arange(NF, dtype=np.int64)
    rsp = ar * NF - (ar * (ar - 1)) // 2
    q = rsp[i_] + (j_ - i_)
    Wd = np.zeros((8128, NF), np.float32)
    Wd[q, k_] = w

    in_maps = []
    for c in range(8):
        big = np.zeros((P, NCOLS), np.float32)
        xs = np.zeros((B, NCLASS), np.float32)
        for t in range(NCLASS):
            i = 8 * t + c
            if i > 126:
                continue
            xs[:, t] = xf[:, i]
            p0 = int(rsp[i])
            # cols j in [i,127) hold Wd rows p0..p0+(127-i); leading j in
            # [8t, i) and trailing j=127 stay zero
            o = int(OFFS[t])
            big[:NF, o + (i - 8 * t): o + (127 - 8 * t)] = Wd[p0:p0 + (NF - i)].T
        m = {"xt": xt, "xb": xb, "xs": xs}
        for ci2, (lo, hi) in enumerate(CHUNKS):
            m[f"wd{ci2}"] = np.ascontiguousarray(
                big[:, int(OFFS[lo]):int(OFFS[hi])])
        in_maps.append(m)
    return in_maps


def _get_nc():
    if "nc" not in _CACHE:
        _CACHE["nc"] = _build_nc()
    return _CACHE["nc"]


def run_spmd(x, weights, comb_idx, trace=False):
    nc = _get_nc()
    in_maps = _prep_inputs(x, weights, comb_idx)
    res = run_bass_kernel_spmd(nc, in_maps, list(range(8)), trace=trace)
    acc = np.zeros((B, 1), np.float64)
    for c in range(8):
        acc += res.results[c]["out"].astype(np.float64)
    return acc.astype(np.float32), res


def kernel(x, weights, comb_idx):
    out, _ = run_spmd(x, weights, comb_idx, trace=False)
    return out



# revision 6
# speedup vs baseline: 1.2958x; 1.2958x over previous
"""HONU order-3 kernel for 8 TRN2 NeuronCores (v2).

Math: out[b] = sum_{i<=j<=k} w_ijk * xf_i * xf_j * xf_k,  xf = [1, x] (127 feats).

Restructuring: for each pair p=(i,j) (i<=j, 8128 pairs) let
    W[k, p] = w_ijk for k in [j,127)  (0 otherwise)
    Z[b, p] = sum_k W[k, p] * xf[b, k]          (dense matmul)
    out[b]  = sum_p Z[b, p] * xf_i(p)[b] * xf_j(p)[b]

The two per-pair factors are HOST-GATHERED index tensors (pure layout, no
arithmetic): XS[b, p] = xf[b, i(p)], XB[b, p] = xf[b, j(p)].  On-device the
whole elementwise stage is then just, per 128-batch tile:
    V1 = Z * XS                      (tensor_tensor)
    res = sum_p (V1 * XB)            (tensor_tensor_reduce, accum_out)

Sharding: pair p -> core p%8 (1016 pairs/core, padded to 1024 columns; two
512-col matmul chunks per batch tile for the PSUM bank limit).  x is
replicated; each core returns a [256]-shaped partial that the host sums.
"""

import os
import numpy as np
import ml_dtypes

import concourse.bass as bass
import concourse.bacc as bacc
import concourse.tile as tile
import concourse.mybir as mybir
from concourse.bass_utils import run_bass_kernel_spmd

F32 = mybir.dt.float32
F32R = mybir.dt.float32r
BF16 = mybir.dt.bfloat16
NPBF16 = ml_dtypes.bfloat16

def _flag(name, default):
    return os.environ.get(name, str(int(default))) == "1"

MM_BF16 = _flag("K_MM_BF16", True)       # bf16 matmul inputs (else f32r)
DVE_BF16 = _flag("K_DVE_BF16", True)     # bf16 xs/xb/v1/scr tensors
USE_TTR = _flag("K_USE_TTR", True)       # tensor_tensor_reduce (else TT+reduce)
USE_TR = _flag("K_USE_TR", True)         # PE-transpose res -> [2,128] out
TT_PSUM = _flag("K_TT_PSUM", True)       # TT1 reads z from PSUM (else ACT copy)

P = 128
NF = 127              # features incl. bias
B = 256               # batch
NPAIR = 8128          # pairs (i<=j), i,j in [0,127)
NCORES = 8
NLOC = NPAIR // NCORES          # 1016 pairs per core
NCOL = 1024                     # padded columns
NCK = 512                       # matmul chunk (PSUM fp32 bank limit)

_CACHE = {}


def _build_nc():
    mm_dt = BF16 if MM_BF16 else F32R
    dve_dt = BF16 if DVE_BF16 else F32
    nc = bacc.Bacc("TRN2", target_bir_lowering=False, debug=False)
    xt = nc.dram_tensor("xt", [P, B], mm_dt, kind="ExternalInput")
    wds = [nc.dram_tensor(f"wd{ck}", [P, NCK], mm_dt, kind="ExternalInput")
           for ck in range(2)]
    xss = [nc.dram_tensor(f"xs{bt}", [P, NCOL], dve_dt, kind="ExternalInput")
           for bt in range(2)]
    xbs = [nc.dram_tensor(f"xb{bt}", [P, NCOL], dve_dt, kind="ExternalInput")
           for bt in range(2)]
    if USE_TR:
        ident = nc.dram_tensor("ident", [P, P], F32, kind="ExternalInput")
        out = nc.dram_tensor("out", [2, P], F32, kind="ExternalOutput")
    else:
        out = nc.dram_tensor("out", [P, 2], F32, kind="ExternalOutput")

    with tile.TileContext(nc) as tc:
        with (
            tc.tile_pool(name="const", bufs=1) as cpool,
            tc.tile_pool(name="ps", bufs=1, space="PSUM") as ps,
        ):
            # --- input DMAs, critical ones first, spread over the two HWDGE
            # queues ---
            xt_t = cpool.tile([P, B], mm_dt, tag="xt")
            nc.sync.dma_start(xt_t[:], xt[:])
            wd_t = [cpool.tile([P, NCK], mm_dt, tag=f"wd{ck}", name=f"wd{ck}_t")
                    for ck in range(2)]
            nc.sync.dma_start(wd_t[0][:], wds[0][:])
            nc.scalar.dma_start(wd_t[1][:], wds[1][:])
            xs_t = [cpool.tile([P, NCOL], dve_dt, tag=f"xs{bt}", name=f"xs{bt}_t")
                    for bt in range(2)]
            xb_t = [cpool.tile([P, NCOL], dve_dt, tag=f"xb{bt}", name=f"xb{bt}_t")
                    for bt in range(2)]
            nc.scalar.dma_start(xs_t[0][:], xss[0][:])
            nc.scalar.dma_start(xb_t[0][:], xbs[0][:])
            nc.sync.dma_start(xs_t[1][:], xss[1][:])
            nc.sync.dma_start(xb_t[1][:], xbs[1][:])
            if USE_TR:
                id_t = cpool.tile([P, P], F32, tag="ident")
                nc.scalar.dma_start(id_t[:], ident[:])

            res_t = cpool.tile([P, 2], F32, tag="res")
            scr_t = cpool.tile([P, NCOL], dve_dt, tag="scr")
            for bt in range(2):
                v1 = cpool.tile([P, NCOL], dve_dt, tag=f"v1_{bt}", name=f"v1_{bt}")
                for ck in range(2):
                    z_ps = ps.tile([P, NCK], F32, tag=f"z{bt}{ck}",
                                   name=f"z{bt}{ck}_ps")
                    nc.tensor.matmul(
                        z_ps[:], xt_t[:, bt * P:(bt + 1) * P], wd_t[ck][:],
                        start=True, stop=True,
                    )
                    if TT_PSUM:
                        z_in = z_ps
                    else:
                        z_sb = cpool.tile([P, NCK], F32, tag=f"zsb{bt}{ck}",
                                          name=f"zsb{bt}{ck}")
                        nc.scalar.copy(z_sb[:], z_ps[:])
                        z_in = z_sb
                    nc.vector.tensor_tensor(
                        v1[:, ck * NCK:(ck + 1) * NCK],
                        z_in[:],
                        xs_t[bt][:, ck * NCK:(ck + 1) * NCK],
                        mybir.AluOpType.mult,
                    )
                if USE_TTR:
                    # fused multiply+row-accumulate via TensorScalarPtr
                    # (tensor_tensor_reduce miscompiles for HW; STT is proven)
                    nc.vector.scalar_tensor_tensor(
                        out=scr_t[:],
                        in0=v1[:],
                        scalar=1.0,
                        in1=xb_t[bt][:],
                        op0=mybir.AluOpType.mult,
                        op1=mybir.AluOpType.mult,
                        accum_out=res_t[:, bt:bt + 1],
                    )
                else:
                    nc.vector.tensor_tensor(
                        scr_t[:], v1[:], xb_t[bt][:], mybir.AluOpType.mult)
                    nc.vector.reduce_sum(
                        res_t[:, bt:bt + 1], scr_t[:], axis=mybir.AxisListType.X)

            if USE_TR:
                # transpose [128,2] -> [2,128] on the PE so the output DMA is
                # two contiguous 512B descriptors instead of 128 8B ones
                tr_ps = ps.tile([2, P], F32, tag="tr")
                nc.tensor.transpose(tr_ps[:], res_t[:], id_t[:])
                stage = cpool.tile([2, P], F32, tag="stage")
                nc.vector.tensor_copy(stage[:], tr_ps[:])
                nc.sync.dma_start(out[:], stage[:])
            else:
                nc.sync.dma_start(out[:], res_t[:])
    nc.compile()
    return nc


def _pair_maps():
    # lex-ordered pairs (i<=j): p = rsp2[i] + (j - i)
    i_of = np.repeat(np.arange(NF), NF - np.arange(NF))
    j_of = np.concatenate([np.arange(i, NF) for i in range(NF)])
    return i_of, j_of


def _prep_inputs(x, weights, comb_idx):
    """Host-side layout prep: gathers/scatters only, no arithmetic on x."""
    mm_npdt = NPBF16 if MM_BF16 else np.float32
    dve_npdt = NPBF16 if DVE_BF16 else np.float32
    x = np.ascontiguousarray(np.asarray(x, dtype=np.float32))
    w = np.asarray(weights, dtype=np.float32).ravel()
    ci = np.asarray(comb_idx)
    i_, j_, k_ = (ci[:, 0].astype(np.int64), ci[:, 1].astype(np.int64),
                  ci[:, 2].astype(np.int64))

    xf = np.concatenate([np.ones((B, 1), np.float32), x], axis=1)  # [256,127]

    ar = np.arange(NF, dtype=np.int64)
    rsp2 = ar * NF - (ar * (ar - 1)) // 2
    p_of_c = rsp2[i_] + (j_ - i_)          # pair index of each triple
    W_all = np.zeros((P, NPAIR), np.float32)
    W_all[k_, p_of_c] = w

    i_of, j_of = _pair_maps()
    xf_d = xf.astype(dve_npdt)
    xt = np.zeros((P, B), mm_npdt)
    xt[:NF, :] = xf.astype(mm_npdt).T

    in_maps = []
    for c in range(NCORES):
        ps_ = np.arange(c, NPAIR, NCORES)
        wd = np.zeros((P, NCOL), mm_npdt)
        wd[:, :NLOC] = W_all[:, ps_].astype(mm_npdt)
        ic, jc = i_of[ps_], j_of[ps_]
        m = {"xt": xt}
        if USE_TR:
            m["ident"] = np.eye(P, dtype=np.float32)
        for ck in range(2):
            m[f"wd{ck}"] = np.ascontiguousarray(wd[:, ck * NCK:(ck + 1) * NCK])
        for bt in range(2):
            xs = np.zeros((P, NCOL), dve_npdt)
            xb = np.zeros((P, NCOL), dve_npdt)
            xs[:, :NLOC] = xf_d[bt * P:(bt + 1) * P, :][:, ic]
            xb[:, :NLOC] = xf_d[bt * P:(bt + 1) * P, :][:, jc]
            m[f"xs{bt}"] = xs
            m[f"xb{bt}"] = xb
        in_maps.append(m)
    return in_maps


def _get_nc():
    if "nc" not in _CACHE:
        _CACHE["nc"] = _build_nc()
    return _CACHE["nc"]


def run_spmd(x, weights, comb_idx, trace=False):
    nc = _get_nc()
    in_maps = _prep_inputs(x, weights, comb_idx)
    res = run_bass_kernel_spmd(nc, in_maps, list(range(NCORES)), trace=trace)
    acc = np.zeros(B, np.float64)
    for c in range(NCORES):
        r = res.results[c]["out"]
        if not USE_TR:
            r = r.T
        acc += r.astype(np.float64).reshape(B)
    return acc[:, None].astype(np.float32), res


def kernel(x, weights, comb_idx):
    out, _ = run_spmd(x, weights, comb_idx, trace=False)
    return out
